# revision 1
# baseline (speedup 1.0000x reference)
"""Trainium2 Bass kernel for nn_AgentModule (multi-agent GRU game module).

Strategy:
 - Shard agent axis A=128 across 8 cores (a=16 agents/core).
 - Feature-major ("transposed") layout on device: h^T [H, rows] so the GRU
   hidden matmuls need no per-step transposes (PE contracts over partitions).
 - fp32 storage for recurrent state + matmul operands, fp32r matmul mode
   (full rate at N=512), bf16 for gate/scratch tensors that never feed a
   matmul (2x DVE).
 - ELU computed as shifted m = elu(x)+1 = min(exp(x), relu(x)+1); the -1 is
   folded into downstream bias vectors (b' = b - W.sum(axis=1)) host-side.
 - GRU input-term gi = x @ Wih^T is static across time: r,z parts are folded
   into the PSUM accumulation as an extra K-chunk; the n part (which must
   stay outside the r* product) is precomputed host-side and streamed.
 - Per-step cost contributions via ACT Square accum_out / tensor_tensor_reduce
   into a [128, 32] column buffer; final dot with ones via PE.
"""

import sys

for _p in ("/opt/trn_rl_repo", "/opt/pypackages"):
    if _p not in sys.path:
        sys.path.append(_p)

import numpy as np
import ml_dtypes

import concourse.bass as bass
import concourse.bacc as bacc
import concourse.mybir as mybir
import concourse.tile as tile
from concourse.bass_utils import run_bass_kernel_spmd

F32 = mybir.dt.float32
F32R = mybir.dt.float32r
BF16 = mybir.dt.bfloat16
AF = mybir.ActivationFunctionType
OP = mybir.AluOpType
AX = mybir.AxisListType

NCORES = 8
A = 128
L = 128
E = A + L          # 256
T = 8
H = 256
V = 32
GS = 5
GI = 5
OD = 2
PD = 3
MD = 2
STEP = 0.1
EPS = 1e-20
AC = A // NCORES   # 16 agents per core
NU = AC * A        # 2048 utterance rows per core
NP = AC * E        # 4096 physical rows per core
W = 1024           # processing block width (free dim)
NBIAS = 112        # bias-matrix columns

# bias column indices (host packing and device use must agree)
BRZ_U, BHN_U, BFC_U = 0, 4, 6
BGP1, BGP2 = 8, 10
BRZ_P, BHN_P, BFC_P = 11, 15, 17
BRZ_G, BHN_G, BFC_G = 19, 23, 25
BRZ_A, BIN_A, BHN_A, BFC_A = 27, 31, 33, 35
BM1, BM2 = 37, 39
BUC1 = 40
BONEC = 42     # column of ones (128 rows)
BNRZ_G = 91    # negated g_bhh r,z bias (4 cols)
BNRZ_A = 95    # negated (a_bih'+a_bhh) r,z bias (4 cols)
BUC2R = 43     # row-0 cols 43:75 = uc2 adjusted bias (as a [1,V] row)
BONER = 75     # row-0 cols 75:91 = ones row (as a [1,16] row)

_DRAM_SPECS = [
    # name, shape, dtype
    ("memuT", (H, NU), BF16),
    ("mempT", (H, NP), BF16),
    ("memaT", (H, AC), F32),
    ("xuT", (V, NU), BF16),
    ("xpT", (OD + PD, NP), BF16),
    ("ginU", (H, NU), BF16),
    ("ginP", (H, NP), BF16),
    ("ggiT", (3 * H, AC), F32),
    ("gumb", (AC, T * V), F32),
    ("whhU", (H, 3 * H), BF16),
    ("wihU", (V, 2 * H), BF16),
    ("whhP", (H, 3 * H), BF16),
    ("wihP", (OD + PD, 2 * H), BF16),
    ("wfcU", (H, H), BF16),
    ("wfcP", (H, H), BF16),
    ("wgp1", (H, H), BF16),
    ("wgp2", (H, GS), BF16),
    ("whhG", (H, 3 * H), F32),
    ("wfcG", (H, H), F32),
    ("wihA", (3 * H, 3 * H), BF16),
    ("whhA", (H, 3 * H), F32),
    ("wfcA", (H, H), F32),
    ("wm1", (H, H), F32),
    ("wm2", (H, MD), F32),
    ("wuc1", (H, H), F32),
    ("wuc2", (H, V), F32),
    ("biases", (128, NBIAS), F32),
    ("bhrows", (1, 512), BF16),
    ("onesw", (1, 512), BF16),
]


def _emit(tc, D, cost_out):
    nc = tc.nc
    import contextlib

    stack = contextlib.ExitStack()
    pers = stack.enter_context(tc.tile_pool(name="pers", bufs=1))

    def persist(name, shape, dtype=F32):
        return pers.tile(list(shape), dtype, tag=name, name=name)

    def load2(name, rows, cols, dtype=F32, ptile=128):
        """Load a [rows, cols] dram tensor as list of [ptile, cols] tiles."""
        nt = (rows + ptile - 1) // ptile
        out = []
        for k in range(nt):
            p = min(ptile, rows - k * ptile)
            tl = persist(f"{name}_{k}", (p, cols), dtype)
            nc.sync.dma_start(tl[:], D[name][k * ptile : k * ptile + p, :])
            out.append(tl)
        return out

    # ---------------- load persistent state + weights ----------------
    hU = load2("memuT", H, NU, dtype=BF16)
    hP = load2("mempT", H, NP, dtype=BF16)
    hA = load2("memaT", H, AC)          # 2 x [128, 16]
    ggiT = load2("ggiT", 3 * H, AC)     # 6 x [128, 16]
    gumb = load2("gumb", AC, T * V)[0]  # [16, 256]

    whhU = load2("whhU", H, 3 * H, dtype=BF16)
    wihU = load2("wihU", V, 2 * H, dtype=BF16)[0]
    whhP = load2("whhP", H, 3 * H, dtype=BF16)
    wihP = load2("wihP", OD + PD, 2 * H, dtype=BF16)[0]
    wfcU = load2("wfcU", H, H, dtype=BF16)
    wfcP = load2("wfcP", H, H, dtype=BF16)
    wgp1 = load2("wgp1", H, H, dtype=BF16)
    wgp2 = load2("wgp2", H, GS, dtype=BF16)
    whhG = load2("whhG", H, 3 * H)
    wfcG = load2("wfcG", H, H)
    wihA = load2("wihA", 3 * H, 3 * H, dtype=BF16)
    whhA = load2("whhA", H, 3 * H)
    wfcA = load2("wfcA", H, H)
    wm1 = load2("wm1", H, H)
    wm2 = load2("wm2", H, MD)
    wuc1 = load2("wuc1", H, H)
    wuc2 = load2("wuc2", H, V)
    bia = load2("biases", 128, NBIAS)[0]
    bhrows = load2("bhrows", 1, 512, dtype=BF16)[0]
    onesw = load2("onesw", 1, 512, dtype=BF16)[0]

    def bvec(idx, p=128):
        return bia[:p, idx : idx + 1]

    cost_buf = persist("cost_buf", (128, 8 * T))
    nc.vector.memset(cost_buf[:], 0.0)

    # ---------------- pools ----------------
    psA = stack.enter_context(tc.tile_pool(name="psA", bufs=4, space="PSUM"))
    psB = stack.enter_context(tc.tile_pool(name="psB", bufs=2, space="PSUM"))
    p_rz = stack.enter_context(tc.tile_pool(name="p_rz", bufs=6))
    p_n = stack.enter_context(tc.tile_pool(name="p_n", bufs=2))
    p_t1 = stack.enter_context(tc.tile_pool(name="p_t1", bufs=2))
    p_s1 = stack.enter_context(tc.tile_pool(name="p_s1", bufs=4))
    p_er = stack.enter_context(tc.tile_pool(name="p_er", bufs=4))
    p_proc = stack.enter_context(tc.tile_pool(name="p_proc", bufs=2))
    p_y1 = stack.enter_context(tc.tile_pool(name="p_y1", bufs=2))
    p_gin = stack.enter_context(tc.tile_pool(name="p_gin", bufs=3))
    p_x = stack.enter_context(tc.tile_pool(name="p_x", bufs=3))
    p_sq = stack.enter_context(tc.tile_pool(name="p_sq", bufs=2))
    p_small = stack.enter_context(tc.tile_pool(name="p_small", bufs=3))
    p_sums = stack.enter_context(tc.tile_pool(name="p_sums", bufs=2))

    def mm(out, lhsT, rhs, start, stop):
        nc.tensor.matmul(out, lhsT, rhs, start=start, stop=stop)

    def mmb(out, lhsT, rhs, start, stop):
        nc.tensor.matmul(out, lhsT, rhs, start=start, stop=stop)

    def gates_psum(whh, wih, hT, xT, m, c0, width, with_x, hc0=None):
        """Gate M-tile m as a list of [128,512] one-bank PSUM sub-tiles
        (deeper PE/consumer pipelining than one multi-bank tile)."""
        if hc0 is None:
            hc0 = c0
        subs = []
        nsub = width // 512
        for s in range(nsub):
            o = psA.tile([128, 512], F32, tag="gates", name=f"gps_{m}_{s}")
            hcc = hc0 + s * 512
            cc = c0 + s * 512
            mm(o[:], whh[0][:, m * 128 : m * 128 + 128], hT[0][:, hcc : hcc + 512], True, False)
            mm(o[:], whh[1][:, m * 128 : m * 128 + 128], hT[1][:, hcc : hcc + 512], False, not with_x)
            if with_x:
                mmb(o[:], wih[:, m * 128 : m * 128 + 128], xT[:, cc : cc + 512], False, True)
            subs.append(o)
        return subs

    def fc_psum(wfc, rhs_tiles, mf, c0, width, pool=None, tag="fc"):
        """PSUM [128, width] = wfc-Mtile-mf applied to rhs (2 K chunks)."""
        ps = (pool or psB).tile([128, width], F32, tag=tag, name=f"fcps_{mf}")
        nsub = width // 512
        for s in range(nsub):
            o = ps[:, s * 512 : (s + 1) * 512]
            cc = c0 + s * 512
            mm(o, wfc[0][:, mf * 128 : mf * 128 + 128], rhs_tiles[0][:, cc : cc + 512], True, False)
            mm(o, wfc[1][:, mf * 128 : mf * 128 + 128], rhs_tiles[1][:, cc : cc + 512], False, True)
        return ps

    def elu_shift(ps, bidx, width, out_pool, out_tag, out_dtype=F32, p=128):
        """m = min(exp(x+b), relu(x+b)+1) from PSUM -> SBUF (= elu(x+b)+1)."""
        e1 = p_er.tile([p, width], BF16, tag="e1")
        nc.scalar.activation(e1[:], ps[:], AF.Exp, bias=bvec(bidx, p))
        r1 = p_er.tile([p, width], BF16, tag="r1")
        nc.scalar.activation(r1[:], ps[:], AF.Relu, bias=bvec(bidx, p))
        m_ = out_pool.tile([p, width], out_dtype, tag=out_tag)
        nc.vector.scalar_tensor_tensor(m_[:], r1[:], 1.0, e1[:], OP.add, OP.min)
        return m_

    def big_gru_gates(rows, hT, ginT_dram, whh, wih, x_dram, xrows, brz,
                      bhn, bh_off, blend_eng, step):
        """Gates + blend for one time-step. Gate tensors are kept in raw
        tanh form t = tanh(x/2 + b/2); the 0.5(1+t) sigmoid affine is folded
        into the downstream STT chains (2*sigma = 1 + t)."""
        nwb = rows // W
        for wb in range(nwb):
            c0 = wb * W
            cols = slice(c0, c0 + W)
            xt = p_x.tile([xrows, W], BF16, tag="x", name=f"xt_{step}_{wb}")
            nc.sync.dma_start(xt[:], x_dram[:, cols])
            rz = []
            for m in range(4):
                subs = gates_psum(whh, wih, hT, xt, m, 0, W, with_x=True, hc0=c0)
                r_ = p_rz.tile([128, W], BF16, tag="rz")
                for s, o in enumerate(subs):
                    nc.scalar.activation(r_[:, s * 512 : (s + 1) * 512], o[:],
                                         AF.Tanh, bias=bvec(brz + m), scale=0.5)
                rz.append(r_)
            ns = []
            for k in range(2):
                m = 4 + k
                subs = gates_psum(whh, wih, hT, None, m, 0, W, with_x=False,
                                  hc0=c0)
                gin = p_gin.tile([128, W], BF16, tag="gin")
                nc.sync.dma_start(gin[:], ginT_dram[k * 128 : k * 128 + 128, cols])
                rfull = p_er.tile([128, W], BF16, tag="rf")
                nc.vector.tensor_scalar(rfull[:], rz[k][:], 0.5, 0.5,
                                        OP.mult, OP.add)
                t1 = p_t1.tile([128, W], BF16, tag="t1")
                for s, o in enumerate(subs):
                    sl = slice(s * 512, (s + 1) * 512)
                    nc.vector.scalar_tensor_tensor(
                        t1[:, sl], o[:], bvec(bhn + k), rfull[:, sl],
                        OP.add, OP.mult
                    )
                t2 = p_s1.tile([128, W], BF16, tag="s1")
                nc.vector.tensor_add(t2[:], t1[:], gin[:])
                n_ = p_n.tile([128, W], BF16, tag="n")
                nc.scalar.activation(n_[:], t2[:], AF.Tanh)
                ns.append(n_)
            for k in range(2):
                # h' = n + z*(h-n) = n + 0.5*(tz+1)*(h-n)
                hk = hT[k][:, cols]
                d = p_s1.tile([128, W], BF16, tag="s1")
                nc.gpsimd.tensor_sub(d[:], hk, ns[k][:])
                e_ = p_s1.tile([128, W], BF16, tag="s1")
                nc.vector.scalar_tensor_tensor(
                    e_[:], rz[2 + k][:], 1.0, d[:], OP.add, OP.mult
                )
                nc.vector.scalar_tensor_tensor(
                    hk, e_[:], 0.5, ns[k][:], OP.mult, OP.add
                )

    def big_gru_fc(rows, hT, bfc, wfc, group, sums, do_gp, step):
        """fc -> shifted ELU -> sums (+ goal-pred cost) (exp table set)."""
        nwb = rows // W
        nag = W // group
        for wb in range(nwb):
            c0 = wb * W
            proc = []
            for mf in range(2):
                ps = fc_psum(wfc, hT, mf, c0, W)
                m_ = elu_shift(ps, bfc + mf, W, p_proc, "proc", BF16)
                proc.append(m_)
                red_in = m_[:].rearrange("p (i j) -> p i j", j=group)
                nc.vector.reduce_sum(
                    sums[mf][:, wb * nag : wb * nag + nag], red_in, axis=AX.X
                )
            if do_gp:
                y1 = []
                for mf in range(2):
                    ps = fc_psum(wgp1, proc, mf, 0, W, tag="fc")
                    y1.append(elu_shift(ps, BGP1 + mf, W, p_y1, "y1", BF16))
                ps5 = psB.tile([GS, W], F32, tag="fc")
                for s in range(W // 512):
                    o = ps5[:, s * 512 : (s + 1) * 512]
                    mm(o, wgp2[0][:, :], y1[0][:, s * 512 : (s + 1) * 512], True, False)
                    mm(o, wgp2[1][:, :], y1[1][:, s * 512 : (s + 1) * 512], False, True)
                sq = p_sq.tile([GS, W], BF16, tag="sq")
                nc.scalar.activation(
                    sq[:], ps5[:], AF.Square, bias=bvec(BGP2, GS),
                    accum_out=cost_buf[:GS, 8 * step + wb : 8 * step + wb + 1],
                )

    # small helpers for the action module ([*, 16] tiles)
    def small_gru_gates(whh, hT, m, extra_k=None):
        ps = psB.tile([128, AC], F32, tag="fc")
        first = True
        if extra_k is not None:
            for ki, rhs in enumerate(extra_k):
                mmb(ps[:], wihA[ki][:, m * 128 : m * 128 + 128], rhs[:], first, False)
                first = False
        mm(ps[:], whh[0][:, m * 128 : m * 128 + 128], hT[0][:], first, False)
        mm(ps[:], whh[1][:, m * 128 : m * 128 + 128], hT[1][:], False, True)
        return ps

    def small_fc(wfc, rhs, tag="fc"):
        out = []
        for mf in range(2):
            ps = psB.tile([128, AC], F32, tag=tag)
            mm(ps[:], wfc[0][:, mf * 128 : mf * 128 + 128], rhs[0][:], True, False)
            mm(ps[:], wfc[1][:, mf * 128 : mf * 128 + 128], rhs[1][:], False, True)
            out.append(ps)
        return out

    def exp_sigmoid(in_ap, hbidx, p_=128, name=None):
        """sigmoid(x+b) = 0.5 + 0.5*tanh(x/2 + b/2) (exp table set)."""
        th = p_small.tile([p_, AC], F32, tag="es", name=name)
        nc.scalar.activation(th[:], in_ap, AF.Tanh, bias=bvec(hbidx, p_), scale=0.5)
        s_ = p_small.tile([p_, AC], BF16, tag="es3", name=(name or "") + "s")
        nc.vector.tensor_scalar(s_[:], th[:], 0.5, 0.5, OP.mult, OP.add)
        return s_

    def small_elu(ps_pair, bidx, tag):
        out = []
        for mf in range(2):
            out.append(
                elu_shift(ps_pair[mf], bidx + mf, AC, p_small, tag, F32)
            )
        return out

    sums_u_prev = None
    for t in range(T):
        sums_u = [
            p_sums.tile([128, AC], F32, tag=f"su{k}", name=f"sums_u{k}_{t}")
            for k in range(2)
        ]
        sums_p = [
            p_sums.tile([128, AC], F32, tag=f"sp{k}", name=f"sums_p{k}_{t}")
            for k in range(2)
        ]
        # phase 1 (sigmoid/tanh table set): gates + blend for both processors
        big_gru_gates(NU, hU, D["ginU"], whhU, wihU, D["xuT"], V, BRZ_U,
                      BHN_U, 0, nc.vector, t)
        big_gru_gates(NP, hP, D["ginP"], whhP, wihP, D["xpT"], OD + PD,
                      BRZ_P, BHN_P, 256, nc.gpsimd, t)
        big_gru_fc(NU, hU, BFC_U, wfcU, A, sums_u, True, t)
        big_gru_fc(NP, hP, BFC_P, wfcP, E, sums_p, False, t)

        # ---- action module ----
        # goal processor GRU (state not persisted)
        grz = []
        for m in range(4):
            ps = small_gru_gates(whhG, hA, m)
            tt = p_small.tile([128, AC], F32, tag="gt")
            nc.vector.tensor_add(tt[:], ps[:], ggiT[m][:])
            grz.append(exp_sigmoid(tt[:], BNRZ_G + m, name=f"grz{m}_{t}"))
        gn = []
        for k in range(2):
            m = 4 + k
            ps = small_gru_gates(whhG, hA, m)
            t1 = p_small.tile([128, AC], BF16, tag="gt")
            nc.vector.scalar_tensor_tensor(
                t1[:], ps[:], bvec(BHN_G + k), grz[k][:], OP.add, OP.mult
            )
            t2 = p_small.tile([128, AC], BF16, tag="gt2")
            nc.vector.tensor_add(t2[:], t1[:], ggiT[m][:])
            n_ = p_small.tile([128, AC], BF16, tag="gn")
            nc.scalar.activation(n_[:], t2[:], AF.Tanh)
            gn.append(n_)
        g2 = []
        for k in range(2):
            d = p_small.tile([128, AC], BF16, tag="gd")
            nc.vector.tensor_sub(d[:], hA[k][:], gn[k][:])
            e_ = p_small.tile([128, AC], BF16, tag="ge")
            nc.vector.tensor_mul(e_[:], grz[2 + k][:], d[:])
            g2k = p_small.tile([128, AC], F32, tag="g2")
            nc.vector.tensor_add(g2k[:], gn[k][:], e_[:])
            g2.append(g2k)
        mg = small_elu(small_fc(wfcG, g2), BFC_G, "mg")

        # action GRU over x = [sums_p/E, sums_u/A, goal_processed] (host-scaled W)
        xch = []
        for xi, src_t in enumerate(
            [sums_p[0], sums_p[1], sums_u[0], sums_u[1], mg[0], mg[1]]
        ):
            xb = p_small.tile([128, AC], BF16, tag=f"xb{xi}", name=f"xb{xi}_{t}")
            nc.vector.tensor_copy(xb[:], src_t[:])
            xch.append(xb)
        arz = []
        for m in range(4):
            ps = small_gru_gates(whhA, hA, m, extra_k=xch)
            arz.append(exp_sigmoid(ps[:], BNRZ_A + m, name=f"arz{m}_{t}"))
        an = []
        for k in range(2):
            m = 4 + k
            psg = psB.tile([128, AC], F32, tag="fc")
            for ki, rhs in enumerate(xch):
                mmb(psg[:], wihA[ki][:, m * 128 : m * 128 + 128], rhs[:],
                    ki == 0, ki == 5)
            psh = psB.tile([128, AC], F32, tag="fc")
            mm(psh[:], whhA[0][:, m * 128 : m * 128 + 128], hA[0][:], True, False)
            mm(psh[:], whhA[1][:, m * 128 : m * 128 + 128], hA[1][:], False, True)
            t1 = p_small.tile([128, AC], BF16, tag="at1")
            nc.vector.scalar_tensor_tensor(
                t1[:], psh[:], bvec(BHN_A + k), arz[k][:], OP.add, OP.mult
            )
            t2 = p_small.tile([128, AC], F32, tag="at2")
            nc.vector.scalar_tensor_tensor(
                t2[:], psg[:], bvec(BIN_A + k), t1[:], OP.add, OP.add
            )
            n_ = p_small.tile([128, AC], BF16, tag="an")
            nc.scalar.activation(n_[:], t2[:], AF.Tanh)
            an.append(n_)
        for k in range(2):
            d = p_small.tile([128, AC], BF16, tag="ad")
            nc.vector.tensor_sub(d[:], hA[k][:], an[k][:])
            e_ = p_small.tile([128, AC], BF16, tag="ae")
            nc.vector.tensor_mul(e_[:], arz[2 + k][:], d[:])
            nc.vector.tensor_add(hA[k][:], an[k][:], e_[:])
        mp = small_elu(small_fc(wfcA, hA), BFC_A, "mp")

        # movement chooser -> cost
        ym = small_elu(small_fc(wm1, mp), BM1, "ym")
        psmv = psB.tile([MD, AC], F32, tag="fc")
        mm(psmv[:], wm2[0][:, :], ym[0][:], True, False)
        mm(psmv[:], wm2[1][:, :], ym[1][:], False, True)
        mvt = p_small.tile([MD, AC], F32, tag="mvt")
        nc.scalar.activation(mvt[:], psmv[:], AF.Tanh, bias=bvec(BM2, MD))
        fmv = p_small.tile([MD, AC], F32, tag="fmv")
        nc.vector.tensor_scalar(fmv[:], mvt[:], 2.0 * STEP, -STEP, OP.mult, OP.add)
        scr = p_small.tile([MD, AC], F32, tag="scr")
        nc.scalar.activation(
            scr[:], fmv[:], AF.Square,
            accum_out=cost_buf[:MD, 8 * t + 5 : 8 * t + 6],
        )

        # utterance chooser -> gumbel softmax -> cost
        yu = small_elu(small_fc(wuc1, mp), BUC1, "yu")
        pslg = psB.tile([AC, V], F32, tag="fc")
        mm(pslg[:], yu[0][:], wuc2[0][:, :], True, False)
        mm(pslg[:], yu[1][:], wuc2[1][:, :], False, False)
        mm(pslg[:], bia[:1, BONER : BONER + AC], bia[:1, BUC2R : BUC2R + V], False, True)
        yg = p_small.tile([AC, V], F32, tag="yg")
        nc.vector.tensor_add(yg[:], pslg[:], gumb[:, t * V : (t + 1) * V])
        nmx = p_small.tile([AC, 1], F32, tag="nmx")
        nc.vector.tensor_reduce(nmx[:], yg[:], axis=AX.X, op=OP.max, negate=True)
        S = p_small.tile([AC, 1], F32, tag="S")
        eu = p_small.tile([AC, V], F32, tag="eu")
        nc.scalar.activation(eu[:], yg[:], AF.Exp, bias=nmx[:], accum_out=S[:])
        q = p_small.tile([AC, 1], F32, tag="q")
        scr2 = p_small.tile([AC, V], F32, tag="scr2")
        nc.scalar.activation(scr2[:], eu[:], AF.Square, accum_out=q[:])
        rs = p_small.tile([AC, 1], F32, tag="rs")
        nc.vector.reciprocal(rs[:], S[:])
        tq = p_small.tile([AC, 1], F32, tag="tq")
        nc.vector.tensor_mul(tq[:], q[:], rs[:])
        nc.vector.tensor_mul(cost_buf[:AC, 8 * t + 4 : 8 * t + 5], tq[:], rs[:])

    # ---------------- final cost reduction ----------------
    csum = persist("csum", (128, 1))
    nc.vector.reduce_sum(csum[:], cost_buf[:], axis=AX.X)
    psc = psB.tile([1, 1], F32, tag="fc")
    mm(psc[:], bia[:, BONEC : BONEC + 1], csum[:], True, True)
    cost_sb = persist("cost_sb", (1, 1))
    nc.scalar.copy(cost_sb[:], psc[:])
    nc.sync.dma_start(cost_out[:], cost_sb[:])

    stack.close()


_CACHED = None


def _build():
    global _CACHED
    if _CACHED is not None:
        return _CACHED
    nc = bacc.Bacc("TRN2", target_bir_lowering=False, debug=False)
    D = {}
    for name, shape, dt_ in _DRAM_SPECS:
        D[name] = nc.dram_tensor(name, list(shape), dt_, kind="ExternalInput")
    cost_out = nc.dram_tensor("cost", [1, 1], F32, kind="ExternalOutput")
    with tile.TileContext(nc) as tc:
        _emit(tc, D, cost_out)
    nc.compile()
    _CACHED = nc
    return nc


def _host_inputs(core, inputs):
    """Build the per-core input map (all host-side numpy preprocessing)."""
    f32 = np.float32
    i0, i1 = core * AC, (core + 1) * AC
    utter = np.asarray(inputs["utterances"], f32)
    obs = np.asarray(inputs["observations"], f32)[i0:i1]
    phys = np.asarray(inputs["physical"], f32)
    goals = np.asarray(inputs["observed_goals"], f32)[i0:i1]
    memu = np.asarray(inputs["mem_utterance"], f32)[i0:i1]
    memp = np.asarray(inputs["mem_physical"], f32)[i0:i1]
    mema = np.asarray(inputs["mem_action"], f32)[i0:i1]
    gumb = np.asarray(inputs["gumbel_u"], f32)[:, i0:i1]

    g = {k: np.asarray(inputs[k], f32) for k in (
        "u_Wih", "u_Whh", "u_bih", "u_bhh", "u_Wfc", "u_bfc",
        "gp_W1", "gp_b1", "gp_W2", "gp_b2",
        "p_Wih", "p_Whh", "p_bih", "p_bhh", "p_Wfc", "p_bfc",
        "g_Wih", "g_Whh", "g_bih", "g_bhh", "g_Wfc", "g_bfc",
        "a_Wih", "a_Whh", "a_bih", "a_bhh", "a_Wfc", "a_bfc",
        "m_W1", "m_b1", "m_W2", "m_b2", "uc_W1", "uc_b1", "uc_W2", "uc_b2")}

    m = {}
    m["memuT"] = np.ascontiguousarray(memu.reshape(NU, H).T).astype(ml_dtypes.bfloat16)
    m["mempT"] = np.ascontiguousarray(memp.reshape(NP, H).T).astype(ml_dtypes.bfloat16)
    m["memaT"] = np.ascontiguousarray(mema.T)
    m["xuT"] = np.ascontiguousarray(np.tile(utter.T, (1, AC))).astype(ml_dtypes.bfloat16)
    xp = np.concatenate(
        [obs, np.broadcast_to(phys[None], (AC, E, PD))], axis=-1
    ).reshape(NP, OD + PD)
    m["xpT"] = np.ascontiguousarray(xp.T).astype(ml_dtypes.bfloat16)
    gin_u = utter @ g["u_Wih"][2 * H :].T + g["u_bih"][2 * H :]      # [A, H]
    m["ginU"] = np.ascontiguousarray(np.tile(gin_u.T, (1, AC))).astype(
        ml_dtypes.bfloat16
    )
    gin_p = xp @ g["p_Wih"][2 * H :].T + g["p_bih"][2 * H :]          # [NP, H]
    m["ginP"] = np.ascontiguousarray(gin_p.T).astype(ml_dtypes.bfloat16)
    ggi = goals @ g["g_Wih"].T + g["g_bih"]                           # [AC, 3H]
    m["ggiT"] = np.ascontiguousarray(ggi.T)
    G = -np.log(-np.log(gumb + f32(EPS)) + f32(EPS)).astype(f32)      # [T,AC,V]
    m["gumb"] = np.ascontiguousarray(G.transpose(1, 0, 2).reshape(AC, T * V))

    m["whhU"] = np.ascontiguousarray(g["u_Whh"].T).astype(ml_dtypes.bfloat16)
    m["wihU"] = np.ascontiguousarray(g["u_Wih"][: 2 * H].T).astype(ml_dtypes.bfloat16)
    m["whhP"] = np.ascontiguousarray(g["p_Whh"].T).astype(ml_dtypes.bfloat16)
    m["wihP"] = np.ascontiguousarray(g["p_Wih"][: 2 * H].T).astype(ml_dtypes.bfloat16)
    m["wfcU"] = np.ascontiguousarray(g["u_Wfc"].T).astype(ml_dtypes.bfloat16)
    m["wfcP"] = np.ascontiguousarray(g["p_Wfc"].T).astype(ml_dtypes.bfloat16)
    m["wgp1"] = np.ascontiguousarray(g["gp_W1"].T).astype(ml_dtypes.bfloat16)
    m["wgp2"] = np.ascontiguousarray(g["gp_W2"].T).astype(ml_dtypes.bfloat16)
    m["whhG"] = np.ascontiguousarray(g["g_Whh"].T)
    m["wfcG"] = np.ascontiguousarray(g["g_Wfc"].T)
    wihA = g["a_Wih"].copy()
    wihA[:, :H] /= E          # phys_feat = sums/E
    wihA[:, H : 2 * H] /= A   # utt_feat = sums/A
    m["wihA"] = np.ascontiguousarray(wihA.T).astype(ml_dtypes.bfloat16)
    m["whhA"] = np.ascontiguousarray(g["a_Whh"].T)
    m["wfcA"] = np.ascontiguousarray(g["a_Wfc"].T)
    m["wm1"] = np.ascontiguousarray(g["m_W1"].T)
    m["wm2"] = np.ascontiguousarray(g["m_W2"].T)
    m["wuc1"] = np.ascontiguousarray(g["uc_W1"].T)
    m["wuc2"] = np.ascontiguousarray(g["uc_W2"].T)

    bias = np.zeros((128, NBIAS), f32)

    def putb(col, vec):
        vec = np.asarray(vec, f32).ravel()
        nt = (len(vec) + 127) // 128
        for k in range(nt):
            sl = vec[k * 128 : (k + 1) * 128]
            bias[: len(sl), col + k] = sl

    putb(BRZ_U, (g["u_bih"][: 2 * H] + g["u_bhh"][: 2 * H]) / 2)
    putb(BHN_U, g["u_bhh"][2 * H :])
    putb(BFC_U, g["u_bfc"])
    putb(BGP1, g["gp_b1"] - g["gp_W1"].sum(1))
    putb(BGP2, g["gp_b2"] - g["gp_W2"].sum(1))
    putb(BRZ_P, (g["p_bih"][: 2 * H] + g["p_bhh"][: 2 * H]) / 2)
    putb(BHN_P, g["p_bhh"][2 * H :])
    putb(BFC_P, g["p_bfc"])
    putb(BRZ_G, g["g_bhh"][: 2 * H])
    putb(BHN_G, g["g_bhh"][2 * H :])
    putb(BFC_G, g["g_bfc"])
    a_bih_adj = g["a_bih"] - g["a_Wih"].sum(1)
    putb(BRZ_A, a_bih_adj[: 2 * H] + g["a_bhh"][: 2 * H])
    putb(BIN_A, a_bih_adj[2 * H :])
    putb(BHN_A, g["a_bhh"][2 * H :])
    putb(BFC_A, g["a_bfc"])
    putb(BM1, g["m_b1"] - g["m_W1"].sum(1))
    putb(BM2, g["m_b2"] - g["m_W2"].sum(1))
    putb(BUC1, g["uc_b1"] - g["uc_W1"].sum(1))
    putb(BNRZ_G, g["g_bhh"][: 2 * H] / 2)
    putb(BNRZ_A, (a_bih_adj[: 2 * H] + g["a_bhh"][: 2 * H]) / 2)
    bias[:, BONEC] = 1.0
    bias[0, BUC2R : BUC2R + V] = (g["uc_b2"] - g["uc_W2"].sum(1)).astype(f32)
    bias[0, BONER : BONER + AC] = 1.0
    m["biases"] = bias
    bhr = np.zeros((1, 512), np.float32)
    bhr[0, 0:256] = g["u_bhh"][2 * H :]
    bhr[0, 256:512] = g["p_bhh"][2 * H :]
    m["bhrows"] = bhr.astype(ml_dtypes.bfloat16)
    m["onesw"] = np.ones((1, 512), ml_dtypes.bfloat16)
    return m


def kernel(**inputs) -> np.ndarray:
    nc = _build()
    in_maps = [_host_inputs(c, inputs) for c in range(NCORES)]
    res = run_bass_kernel_spmd(nc, in_maps, core_ids=list(range(NCORES)))
    total = np.float32(0.0)
    for r in res.results:
        total += np.float32(r["cost"].reshape(-1)[0])
    return np.array([total], np.float32)


if __name__ == "__main__":
    _build()
    print("build ok")

